# revision 31
# baseline (speedup 1.0000x reference)
"""Tensor-parallel causal multi-head attention (RoPE) for 8 Trainium2 cores.

Problem: nn_Attention (B=2, S=2048, E=2048, H=16, interleaved-pair RoPE,
causal softmax with 1/sqrt(E) scaling, output projection).

Sharding: tensor-parallel over heads — each of the 8 cores owns 2 heads
(the matching 256 columns of Wq/Wk/Wv and rows of Wo), x is replicated,
and the post-Wo all-reduce is done on the host (sum of 8 partials).

Per-core device pipeline (all matmuls bf16, fp32 accumulation):
  1. QK^T projections in transposed layout  Q^T/K^T [D, t]  (tokens on the
     free axis), V in natural layout [t, D].  RoPE is applied to Q^T/K^T on
     the vector engine using host-precomputed cos/sin maps; the head-dim is
     de-interleaved (even feats then odd feats) via a host-side permutation
     of the Wq/Wk rows so the rotation pairs are contiguous partitions.
  2. Attention per (batch, head) over q-tiles of 512 with 128-wide key
     chunks processed in pairs (one [128,1024] exp per pair on ACT when the
     pair is untrimmed or first-diagonal, scale folded in; no
     max-subtraction: |scores/sqrt(E)| <~ 1.5 for these inputs).  Diagonal
     chunks are causally trimmed to their valid N = 512-128*j query range
     and masked with the sliced j=0 triangle mask (DVE).  PV accumulates
     out^T += V_c^T probs^T  on PE into the left bank of a 2-bank psO
     tile; the denominator rides on PE as a ones[128,128]-stationary
     matmul into the right bank.
  3. Normalization: reciprocal_approx_fast(denom) then one DVE multiply
     fused into the out^T psum eviction.
  4. Output projection from out^T (stationary) -> bf16 partial [t, E],
     staged per token-chunk (evictions alternate DVE/ACT so neither engine
     gates PE) and written back with one DMA.  Host sums the 8 partials
     in fp64.

Scheduling notes (measured, HW exec ~350 us vs 385 us predecessor):
  - Phase emission A(0..7), B0, B1, C0, C1: B1's scores depend only on A
    so they fill B0's exp-wait gaps; C0's chunks become ready per-q-tile
    during B and fill B1's; the ACT exp stream runs continuously.
  - Fine-grained A/B interleaving and moving the denominator off PE both
    measured WORSE: cross-engine deps starve the PE sequencer's 4-deep
    wait queue (exposed LDWEIGHTS + HAM clock-gate oscillation, up to
    +19%% on matmul pacing).
  - PSUM as two tags x 2 bufs of [128,1024] f32 (all 8 banks): phases A/C
    alternate tags for a 4-deep matmul-psum rotation; phase B uses one
    tag for the double-buffered PV/denominator q-tile chains (consecutive
    chains overlap) and the other for score pairs.
  - Startup: coalesced ~0.5 MB DMAs ordered wq -> x(0) -> wk -> wv (the
    single HWDGE ring is the startup bottleneck), RoPE maps sliced
    per-tile, Wo deferred to mid-phase-A; 48 dummy matmuls bridge the
    startup DMA window so the PE HAM clock stays warm.
"""

import math
import os
from contextlib import ExitStack

import ml_dtypes
import numpy as np

import concourse.bass as bass
import concourse.mybir as mybir
import concourse.tile as tile
from concourse import bacc, bass_isa, bass_utils

# partial-output dtype: bf16 halves the output DMA; host sums in fp64
OUT_BF16 = os.environ.get("KERNEL_OUT", "bf16") == "bf16"

# ---------------------------------------------------------------- constants
B, S, E = 2, 2048, 2048
H = 16
N_CORES = 8
HPC = H // N_CORES          # heads per core = 2
D = E // H                  # head dim = 128
T = B * S                   # tokens = 4096
HD = HPC * D                # per-core head dims = 256
ATTN_SCALE = 1.0 / math.sqrt(E)
ROPE_BASE = 10000.0

P = 128
EC = E // P                 # 16 contraction chunks
T_TILE = 512
NT = T // T_TILE            # 8 projection token tiles
QTS = 512                   # attention q-tile size
NQT = S // QTS              # 4 q-tiles per (b, h)
NKC = S // P                # 16 key chunks per batch
N_WARM = 20                 # HAM warm-up matmuls bridging the startup DMAs

BF16 = mybir.dt.bfloat16
F32 = mybir.dt.float32
NPBF16 = ml_dtypes.bfloat16


# ---------------------------------------------------------------- device IR
def _emit(tc, ctx):
    nc = tc.nc
    xTt = nc.dram_tensor("xTt", [NT, P, EC, T_TILE], BF16, kind="ExternalInput").ap()
    wqT = nc.dram_tensor("wqT", [P, EC, HD], BF16, kind="ExternalInput").ap()
    wkT = nc.dram_tensor("wkT", [P, EC, HD], BF16, kind="ExternalInput").ap()
    wvT = nc.dram_tensor("wvT", [P, EC, HD], BF16, kind="ExternalInput").ap()
    woT = nc.dram_tensor("woT", [P, HPC, E], BF16, kind="ExternalInput").ap()
    rm1 = nc.dram_tensor("rm1", [P, T], BF16, kind="ExternalInput").ap()
    rm2 = nc.dram_tensor("rm2", [P, T], BF16, kind="ExternalInput").ap()
    msk = nc.dram_tensor("msk", [P, 4, QTS], BF16, kind="ExternalInput").ap()
    out = nc.dram_tensor("out", [T, E], BF16 if OUT_BF16 else F32,
                         kind="ExternalOutput").ap()

    wpool = ctx.enter_context(tc.tile_pool(name="wpool", bufs=1))
    xpool = ctx.enter_context(tc.tile_pool(name="xpool", bufs=2))
    qkv = ctx.enter_context(tc.tile_pool(name="qkv", bufs=1))
    work = ctx.enter_context(tc.tile_pool(name="work", bufs=3))
    psA = ctx.enter_context(tc.tile_pool(name="psA", bufs=2, space="PSUM"))
    psO = ctx.enter_context(tc.tile_pool(name="psO", bufs=2, space="PSUM"))

    # --- persistent SBUF state
    wq_s = wpool.tile([P, EC, HD], BF16)
    wk_s = wpool.tile([P, EC, HD], BF16)
    wv_s = wpool.tile([P, EC, HD], BF16)
    wo_s = wpool.tile([P, HPC, E], BF16)
    m1_s = wpool.tile([P, T], BF16)
    m2_s = wpool.tile([P, T], BF16)
    mk_s = wpool.tile([P, 4, QTS], BF16)
    ones_s = wpool.tile([P, P], BF16)
    nc.any.memset(ones_s[:], 1.0)

    # startup-latency ordering: feed the first Q-group's operands (all of
    # wq + x tile 0) ahead of everything else, then wk, wv, masks.
    q4sl = [slice(q * (EC // 4), (q + 1) * (EC // 4)) for q in range(4)]
    # startup burst split across BOTH HWDGE rings: weights on the scalar
    # ring (ACT has no compute yet, so no queue interference), the x tile
    # on the sync ring; coalesced ~0.5 MB transfers (per-DMA ring overhead
    # is what delays the later pieces)
    nc.scalar.dma_start(wq_s[:, 0:8, :], wqT[:, 0:8, :])
    xt0 = xpool.tile([P, EC, T_TILE], BF16, tag="xt")
    nc.sync.dma_start(xt0[:, 0:4, :], xTt[0, :, 0:4, :])
    nc.scalar.dma_start(wq_s[:, 8:16, :], wqT[:, 8:16, :])
    nc.sync.dma_start(xt0[:, 4:8, :], xTt[0, :, 4:8, :])
    # short HAM warm-up; real matmuls chase the startup DMAs from ~9.5us
    warm = psA.tile([P, 512], F32, tag="big", bufs=2,
                    padded_shape=[P, 2 * QTS])
    for i in range(N_WARM):
        nc.tensor.matmul(warm[:, 0:P], lhsT=ones_s[:], rhs=ones_s[:],
                         start=(i == 0), stop=(i == N_WARM - 1))
    nc.sync.dma_start(xt0[:, 8:12, :], xTt[0, :, 8:12, :])
    nc.sync.dma_start(xt0[:, 12:16, :], xTt[0, :, 12:16, :])
    for h2 in range(2):
        nc.scalar.dma_start(wk_s[:, 8 * h2:8 * (h2 + 1), :],
                            wkT[:, 8 * h2:8 * (h2 + 1), :])
    for h2 in range(2):
        nc.scalar.dma_start(wv_s[:, 8 * h2:8 * (h2 + 1), :],
                            wvT[:, 8 * h2:8 * (h2 + 1), :])
    nc.scalar.dma_start(mk_s[:], msk[:])

    qT_s = qkv.tile([P, HPC, T], BF16)   # roped Q^T  [d, h, t]
    kT_s = qkv.tile([P, HPC, T], BF16)   # roped K^T
    v_s = qkv.tile([P, T // P, HD], BF16)  # V natural [t%128, t//128, hd]
    oT_s = qkv.tile([P, HPC, T], BF16)   # normalized out^T [d, h, t]

    xt_tiles = {0: xt0}

    # two psum tags, each 2 bufs of [P, 1024] f32 (2 banks) = all 8 banks;
    # phases A/C alternate tags for a 4-deep matmul-psum rotation, phase B
    # uses "outT" for the per-q-tile PV/denominator chains and "big" for
    # score pairs
    def _ps_tile(pool, shape=None):
        tg = "big" if pool is psA else "outT"
        return pool.tile(shape or [P, 2 * QTS], F32, name=tg + "_t",
                         tag=tg, bufs=2, padded_shape=[P, 2 * QTS])

    # ---------------- phase A unit: projections + RoPE for one token tile
    def emit_proj(tt):
        ts0 = tt * T_TILE
        if tt in xt_tiles:
            xt = xt_tiles[tt]
        else:
            xt = xpool.tile([P, EC, T_TILE], BF16, tag="xt")
            for q4 in range(4):
                nc.sync.dma_start(xt[:, q4sl[q4], :], xTt[tt, :, q4sl[q4], :])
        # rope map slices for this token range
        nc.sync.dma_start(m1_s[:, ts0:ts0 + T_TILE], rm1[:, ts0:ts0 + T_TILE])
        nc.sync.dma_start(m2_s[:, ts0:ts0 + T_TILE], rm2[:, ts0:ts0 + T_TILE])

        pools = [psA, psO, psA, psO]  # alternate pools -> 4-deep rotation
        for gi, (w_s, dst) in enumerate(((wq_s, qT_s), (wk_s, kT_s))):
            psb = _ps_tile(pools[gi])
            for hs in range(HPC):
                ps = psb[:, hs * T_TILE:(hs + 1) * T_TILE]
                for ec in range(EC):
                    nc.tensor.matmul(
                        ps,
                        lhsT=w_s[:, ec, hs * P:(hs + 1) * P],
                        rhs=xt[:, ec, :],
                        start=(ec == 0),
                        stop=(ec == EC - 1),
                    )
                # RoPE: e = [x1; x2], swp = [x2; x1] (half-swap via DMA);
                # out = e*[cos;cos] + swp*[-sin;sin]
                e_t = work.tile([P, T_TILE], BF16, tag="rope_e")
                nc.scalar.copy(e_t[:], ps)
                swp = work.tile([P, T_TILE], BF16, tag="rope_s")
                nc.sync.dma_start(swp[0:64, :], e_t[64:128, :])
                nc.sync.dma_start(swp[64:128, :], e_t[0:64, :])
                a_t = work.tile([P, T_TILE], BF16, tag="rope_a")
                b_t = work.tile([P, T_TILE], BF16, tag="rope_b")
                nc.vector.tensor_mul(a_t[:], e_t[:], m1_s[:, ts0:ts0 + T_TILE])
                nc.vector.tensor_mul(b_t[:], swp[:], m2_s[:, ts0:ts0 + T_TILE])
                nc.vector.tensor_add(dst[:, hs, ts0:ts0 + T_TILE], a_t[:], b_t[:])

        for sp in range(T_TILE // P // 2):
            psb = _ps_tile(pools[2 + sp], shape=[P, 2 * HD])
            for k in range(2):
                sub = 2 * sp + k
                for ec in range(EC):
                    nc.tensor.matmul(
                        psb[:, k * HD:(k + 1) * HD],
                        lhsT=xt[:, ec, sub * P:(sub + 1) * P],
                        rhs=wv_s[:, ec, :],
                        start=(ec == 0),
                        stop=(ec == EC - 1),
                    )
            nc.scalar.copy(
                v_s[:, tt * (T_TILE // P) + 2 * sp:
                    tt * (T_TILE // P) + 2 * sp + 2, :], psb[:])

    # ---------------- phase B unit: attention for one (batch, q-tile)
    def emit_attn(b, qt):
        q0 = qt * QTS
        nck = (q0 + QTS) // P  # causal: key chunks 0..nck-1
        for hs in range(HPC):
            qTb = qT_s[:, hs, b * S:(b + 1) * S]
            kTb = kT_s[:, hs, b * S:(b + 1) * S]
            # one 2-bank tile per q-tile: left bank = PV out^T accumulator,
            # right bank = denominator (ones-matmul); double-buffered so
            # consecutive q-tile chains overlap
            opd = _ps_tile(psO)
            ops = opd[:, 0:QTS]
            dps = opd[:, QTS:2 * QTS]
            for pp in range(nck // 2):
                cc = (2 * pp, 2 * pp + 1)
                # causal trim: diagonal chunk j (=c-(nck-4)) only has
                # valid queries q >= q0 + 128*j  ->  width 512-128*j
                jj = [max(0, c - (nck - 4)) for c in cc]
                off = [128 * j for j in jj]
                sps = _ps_tile(psA)
                for half, c in enumerate(cc):
                    nc.tensor.matmul(
                        sps[:, half * QTS + off[half]:(half + 1) * QTS],
                        lhsT=kTb[:, c * P:(c + 1) * P],
                        rhs=qTb[:, q0 + off[half]:q0 + QTS],
                        start=True,
                        stop=True,
                    )
                ex = work.tile([P, 2 * QTS], BF16, tag="exps", bufs=6)
                if off[0] == 0:
                    # single full-width exp; any garbage in half 1's
                    # trimmed zone is finite and never consumed
                    nc.scalar.activation(
                        ex[:], sps[:], mybir.ActivationFunctionType.Exp,
                        scale=ATTN_SCALE,
                    )
                else:
                    for half in range(2):
                        sl = slice(half * QTS + off[half], (half + 1) * QTS)
                        nc.scalar.activation(
                            ex[:, sl], sps[:, sl],
                            mybir.ActivationFunctionType.Exp,
                            scale=ATTN_SCALE,
                        )
                for half, c in enumerate(cc):
                    w = QTS - off[half]
                    exh = ex[:, half * QTS + off[half]:(half + 1) * QTS]
                    if c >= nck - 4:
                        # intra-block triangle: reuse the j=0 mask, width w
                        nc.vector.tensor_mul(exh, exh, mk_s[:, 0, :w])
                    nc.tensor.matmul(
                        ops[:, off[half]:QTS],
                        lhsT=v_s[:, b * NKC + c, hs * P:(hs + 1) * P],
                        rhs=exh,
                        start=(c == 0),
                        stop=(c == nck - 1),
                    )
                    nc.tensor.matmul(
                        dps[:, off[half]:QTS],
                        lhsT=ones_s[:],
                        rhs=exh,
                        start=(c == 0),
                        stop=(c == nck - 1),
                    )
            # normalize: oT = ops * (1/denom), denom replicated to all
            # 128 partitions by the ones-matmul
            oslice = oT_s[:, hs, b * S + q0: b * S + q0 + QTS]
            rb = work.tile([P, QTS], F32, tag="recipb")
            nc.vector.reciprocal_approx_fast(out=rb[:], in_=dps)
            nc.vector.tensor_mul(oslice, ops, rb[:])

    # ---------------- phase C unit: output projection for one batch
    def emit_outproj(b, last):
        for tch in range(S // P):
            t0 = b * S + tch * P
            stage = work.tile([P, E], BF16 if OUT_BF16 else F32, tag="wo_out")
            # hc-outer: one stationary (oT chunk) streams all 4 E-slices
            # before switching, consolidating the PE weight stream
            wps = [_ps_tile(psA, shape=[P, 1024]),
                   _ps_tile(psO, shape=[P, 1024])]
            for hc in range(HPC):
                for es in range(4):
                    nc.tensor.matmul(
                        wps[es // 2][:, (es % 2) * 512:(es % 2 + 1) * 512],
                        lhsT=oT_s[:, hc, t0:t0 + P],
                        rhs=wo_s[:, hc, es * 512:(es + 1) * 512],
                        start=(hc == 0),
                        stop=(hc == HPC - 1),
                    )
            for ep in range(2):
                # evictions alternate DVE/ACT so neither engine gates PE
                if ep == 0:
                    nc.vector.tensor_copy(
                        out=stage[:, 0:1024], in_=wps[0][:])
                else:
                    nc.scalar.copy(stage[:, 1024:2048], wps[1][:])
                if last and tch == S // P - 1:
                    # drain the final tile per-slice to shorten the tail
                    nc.sync.dma_start(
                        out[t0:t0 + P, ep * 1024:(ep + 1) * 1024],
                        stage[:, ep * 1024:(ep + 1) * 1024])
            if not (last and tch == S // P - 1):
                nc.sync.dma_start(out[t0:t0 + P, :], stage[:])

    # ---------------- emission schedule: A, B0, B1, C0, C1.  B1's scores
    # only need phase A, so they fill B0's exp-wait gaps; C0's chunks
    # become ready per-q-tile during B and fill B1's; the ACT exp stream
    # runs continuously across B0+B1.  (Fine-grained A/B interleaving was
    # measured WORSE — PE sequencer wait-queue starvation + HAM.)
    for tt in range(NT):
        emit_proj(tt)
        if tt == 1:
            nc.sync.dma_start(wo_s[:], woT[:])
    for b in range(B):
        for qt in range(NQT):
            emit_attn(b, qt)
    emit_outproj(0, last=False)
    emit_outproj(1, last=True)


def build_nc():
    nc = bacc.Bacc("TRN2", target_bir_lowering=False, debug=False, num_devices=1)
    with tile.TileContext(nc) as tc, ExitStack() as ctx:
        _emit(tc, ctx)
    nc.compile()
    return nc


# ---------------------------------------------------------------- host prep
def _rope_maps():
    half = D // 2
    inv = 1.0 / (ROPE_BASE ** (np.arange(half, dtype=np.float64) / half))
    ang = np.arange(S, dtype=np.float64)[None, :] * inv[:, None]  # [64, S]
    cos = np.cos(ang)
    sin = np.sin(ang)
    m1 = np.concatenate([cos, cos], axis=0)   # [128, S] multiplies e=[x1;x2]
    m2 = np.concatenate([-sin, sin], axis=0)  # multiplies swp=[x2;x1]
    m1 = np.tile(m1, (1, B)).astype(NPBF16)   # [128, T] (t = b*S + s)
    m2 = np.tile(m2, (1, B)).astype(NPBF16)
    return np.ascontiguousarray(m1), np.ascontiguousarray(m2)


def _masks():
    kk = np.arange(P)[:, None]
    qq = np.arange(QTS)[None, :]
    m = np.stack([(kk + 128 * j <= qq) for j in range(4)], axis=1)
    return np.ascontiguousarray(m.astype(NPBF16))  # [128, 4, 512]


def _prep_in_maps(x, Wq, Wk, Wv, Wo):
    x = np.asarray(x, np.float32)
    Wq = np.asarray(Wq, np.float32)
    Wk = np.asarray(Wk, np.float32)
    Wv = np.asarray(Wv, np.float32)
    Wo = np.asarray(Wo, np.float32)

    # x^T tiled: [NT, 128, EC, T_TILE];  xT[e, t] = x[t, e]
    xT = x.reshape(T, E).T.astype(NPBF16)                      # [E, T]
    xtt = xT.reshape(EC, P, NT, T_TILE).transpose(2, 1, 0, 3)  # [NT,P,EC,TT]
    xtt = np.ascontiguousarray(xtt)

    m1, m2 = _rope_maps()
    msk = _masks()

    # de-interleave perm for RoPE pair-contiguity
    perm = np.concatenate([np.arange(0, D, 2), np.arange(1, D, 2)])

    def wslice(W, rows):
        # -> [P, EC, ncols] : wT[p, ec, c] = W[rows[c], ec*128 + p]
        wt = W[rows].T.astype(NPBF16)            # [E, ncols]
        return np.ascontiguousarray(
            wt.reshape(EC, P, len(rows)).transpose(1, 0, 2))

    in_maps = []
    for core in range(N_CORES):
        heads = range(core * HPC, (core + 1) * HPC)
        rows_qk = np.concatenate([h * D + perm for h in heads])
        rows_v = np.concatenate([np.arange(h * D, (h + 1) * D) for h in heads])
        # woT[p, hc, e] = Wo[e, rows_v[hc*128 + p]]
        wo_t = Wo[:, rows_v].T.astype(NPBF16)    # [HD, E]
        wo_t = np.ascontiguousarray(
            wo_t.reshape(HPC, P, E).transpose(1, 0, 2))
        in_maps.append({
            "xTt": xtt,
            "wqT": wslice(Wq, rows_qk),
            "wkT": wslice(Wk, rows_qk),
            "wvT": wslice(Wv, rows_v),
            "woT": wo_t,
            "rm1": m1,
            "rm2": m2,
            "msk": msk,
        })
    return in_maps


_NC_CACHE = None


def _get_nc():
    global _NC_CACHE
    if _NC_CACHE is None:
        _NC_CACHE = build_nc()
    return _NC_CACHE


def kernel(x, Wq, Wk, Wv, Wo, _want_trace=False):
    in_maps = _prep_in_maps(x, Wq, Wk, Wv, Wo)
    nc = _get_nc()
    trace = _want_trace or bool(os.environ.get("KERNEL_TRACE"))
    res = bass_utils.run_bass_kernel_spmd(
        nc, in_maps, core_ids=list(range(N_CORES)), trace=trace,
    )
    acc = np.zeros((T, E), np.float64)
    for c in range(N_CORES):
        acc += res.results[c]["out"].astype(np.float64)
    outv = acc.astype(np.float32).reshape(B, S, E)
    if _want_trace:
        return outv, res
    return outv


# revision 32
# speedup vs baseline: 1.2063x; 1.2063x over previous
"""Tensor-parallel causal multi-head attention (RoPE) for 8 Trainium2 cores.

Problem: nn_Attention (B=2, S=2048, E=2048, H=16, interleaved-pair RoPE,
causal softmax with 1/sqrt(E) scaling, output projection).

Sharding: tensor-parallel over heads — each of the 8 cores owns 2 heads
(the matching 256 columns of Wq/Wk/Wv and rows of Wo), x is replicated,
and the post-Wo all-reduce is done on the host (sum of 8 partials).

Per-core device pipeline (all matmuls bf16, fp32 accumulation):
  1. QK^T projections in transposed layout  Q^T/K^T [D, t]  (tokens on the
     free axis), V in natural layout [t, D].  RoPE is applied to Q^T/K^T on
     the vector engine using host-precomputed cos/sin maps; the head-dim is
     de-interleaved (even feats then odd feats) via a host-side permutation
     of the Wq/Wk rows so the rotation pairs are contiguous partitions.
  2. Attention per (batch, head) over q-tiles of 512 with 128-wide key
     chunks processed in pairs (one [128,1024] exp per pair on ACT when the
     pair is untrimmed or first-diagonal, scale folded in; no
     max-subtraction: |scores/sqrt(E)| <~ 1.5 for these inputs).  Diagonal
     chunks are causally trimmed to their valid N = 512-128*j query range
     and masked with the sliced j=0 triangle mask (DVE).  PV accumulates
     out^T += V_c^T probs^T  on PE into the left bank of a 2-bank psO
     tile; the denominator rides on PE as a ones[128,128]-stationary
     matmul into the right bank.
  3. Normalization: reciprocal_approx_fast(denom) then one DVE multiply
     fused into the out^T psum eviction.
  4. Output projection from out^T (stationary) -> bf16 partial [t, E],
     staged per token-chunk (evictions alternate DVE/ACT so neither engine
     gates PE) and written back with one DMA.  Host sums the 8 partials
     in fp64.

Scheduling notes (measured, HW exec ~350 us vs 385 us predecessor):
  - Phase emission A(0..7), B0, B1, C0, C1: B1's scores depend only on A
    so they fill B0's exp-wait gaps; C0's chunks become ready per-q-tile
    during B and fill B1's; the ACT exp stream runs continuously.
  - Fine-grained A/B interleaving and moving the denominator off PE both
    measured WORSE: cross-engine deps starve the PE sequencer's 4-deep
    wait queue (exposed LDWEIGHTS + HAM clock-gate oscillation, up to
    +19%% on matmul pacing).
  - PSUM as two tags x 2 bufs of [128,1024] f32 (all 8 banks): phases A/C
    alternate tags for a 4-deep matmul-psum rotation; phase B uses one
    tag for the double-buffered PV/denominator q-tile chains (consecutive
    chains overlap) and the other for score pairs.
  - Startup: coalesced ~0.5 MB DMAs ordered wq -> x(0) -> wk -> wv (the
    single HWDGE ring is the startup bottleneck), RoPE maps sliced
    per-tile, Wo deferred to mid-phase-A; 48 dummy matmuls bridge the
    startup DMA window so the PE HAM clock stays warm.
"""

import math
import os
from contextlib import ExitStack

import ml_dtypes
import numpy as np

import concourse.bass as bass
import concourse.mybir as mybir
import concourse.tile as tile
from concourse import bacc, bass_isa, bass_utils

# partial-output dtype: bf16 halves the output DMA; host sums in fp64
OUT_BF16 = os.environ.get("KERNEL_OUT", "bf16") == "bf16"

# ---------------------------------------------------------------- constants
B, S, E = 2, 2048, 2048
H = 16
N_CORES = 8
HPC = H // N_CORES          # heads per core = 2
D = E // H                  # head dim = 128
T = B * S                   # tokens = 4096
HD = HPC * D                # per-core head dims = 256
ATTN_SCALE = 1.0 / math.sqrt(E)
ROPE_BASE = 10000.0

P = 128
EC = E // P                 # 16 contraction chunks
T_TILE = 512
NT = T // T_TILE            # 8 projection token tiles
QTS = 512                   # attention q-tile size
NQT = S // QTS              # 4 q-tiles per (b, h)
NKC = S // P                # 16 key chunks per batch
N_WARM = 48                 # HAM warm-up matmuls bridging the startup DMAs

BF16 = mybir.dt.bfloat16
F32 = mybir.dt.float32
NPBF16 = ml_dtypes.bfloat16


# ---------------------------------------------------------------- device IR
def _emit(tc, ctx):
    nc = tc.nc
    xTt = nc.dram_tensor("xTt", [NT, P, EC, T_TILE], BF16, kind="ExternalInput").ap()
    wqT = nc.dram_tensor("wqT", [P, EC, HD], BF16, kind="ExternalInput").ap()
    wkT = nc.dram_tensor("wkT", [P, EC, HD], BF16, kind="ExternalInput").ap()
    wvT = nc.dram_tensor("wvT", [P, EC, HD], BF16, kind="ExternalInput").ap()
    woT = nc.dram_tensor("woT", [P, HPC, E], BF16, kind="ExternalInput").ap()
    rm1 = nc.dram_tensor("rm1", [P, T], BF16, kind="ExternalInput").ap()
    rm2 = nc.dram_tensor("rm2", [P, T], BF16, kind="ExternalInput").ap()
    msk = nc.dram_tensor("msk", [P, 4, QTS], BF16, kind="ExternalInput").ap()
    out = nc.dram_tensor("out", [T, E], BF16 if OUT_BF16 else F32,
                         kind="ExternalOutput").ap()

    wpool = ctx.enter_context(tc.tile_pool(name="wpool", bufs=1))
    xpool = ctx.enter_context(tc.tile_pool(name="xpool", bufs=2))
    qkv = ctx.enter_context(tc.tile_pool(name="qkv", bufs=1))
    work = ctx.enter_context(tc.tile_pool(name="work", bufs=3))
    psA = ctx.enter_context(tc.tile_pool(name="psA", bufs=2, space="PSUM"))
    psO = ctx.enter_context(tc.tile_pool(name="psO", bufs=2, space="PSUM"))

    # --- persistent SBUF state
    wq_s = wpool.tile([P, EC, HD], BF16)
    wk_s = wpool.tile([P, EC, HD], BF16)
    wv_s = wpool.tile([P, EC, HD], BF16)
    wo_s = wpool.tile([P, HPC, E], BF16)
    m1_s = wpool.tile([P, T], BF16)
    m2_s = wpool.tile([P, T], BF16)
    mk_s = wpool.tile([P, 4, QTS], BF16)
    ones_s = wpool.tile([P, P], BF16)
    nc.any.memset(ones_s[:], 1.0)

    # startup-latency ordering: feed the first Q-group's operands (all of
    # wq + x tile 0) ahead of everything else, then wk, wv, masks.
    q4sl = [slice(q * (EC // 4), (q + 1) * (EC // 4)) for q in range(4)]
    # coalesced transfers (~0.5 MB each) — per-DMA ring overhead is what
    # delays the later weights, so fewer/bigger beats many/small here
    nc.sync.dma_start(wq_s[:, 0:8, :], wqT[:, 0:8, :])
    xt0 = xpool.tile([P, EC, T_TILE], BF16, tag="xt")
    nc.sync.dma_start(xt0[:, 0:4, :], xTt[0, :, 0:4, :])
    nc.sync.dma_start(wq_s[:, 8:16, :], wqT[:, 8:16, :])
    nc.sync.dma_start(xt0[:, 4:8, :], xTt[0, :, 4:8, :])
    # HAM warm-up: dummy matmuls bridging the startup DMA window so the
    # first real matmuls run at the full 2.4 GHz clock
    warm = psA.tile([P, 512], F32, tag="big", bufs=2,
                    padded_shape=[P, 2 * QTS])
    for i in range(N_WARM):
        nc.tensor.matmul(warm[:, 0:P], lhsT=ones_s[:], rhs=ones_s[:],
                         start=(i == 0), stop=(i == N_WARM - 1))
    nc.sync.dma_start(xt0[:, 8:12, :], xTt[0, :, 8:12, :])
    nc.sync.dma_start(xt0[:, 12:16, :], xTt[0, :, 12:16, :])
    for h2 in range(2):
        nc.sync.dma_start(wk_s[:, 8 * h2:8 * (h2 + 1), :],
                          wkT[:, 8 * h2:8 * (h2 + 1), :])
    for h2 in range(2):
        nc.sync.dma_start(wv_s[:, 8 * h2:8 * (h2 + 1), :],
                          wvT[:, 8 * h2:8 * (h2 + 1), :])
    nc.sync.dma_start(mk_s[:], msk[:])

    qT_s = qkv.tile([P, HPC, T], BF16)   # roped Q^T  [d, h, t]
    kT_s = qkv.tile([P, HPC, T], BF16)   # roped K^T
    v_s = qkv.tile([P, T // P, HD], BF16)  # V natural [t%128, t//128, hd]
    oT_s = qkv.tile([P, HPC, T], BF16)   # normalized out^T [d, h, t]

    xt_tiles = {0: xt0}

    # two psum tags, each 2 bufs of [P, 1024] f32 (2 banks) = all 8 banks;
    # phases A/C alternate tags for a 4-deep matmul-psum rotation, phase B
    # uses "outT" for the per-q-tile PV/denominator chains and "big" for
    # score pairs
    def _ps_tile(pool, shape=None):
        tg = "big" if pool is psA else "outT"
        return pool.tile(shape or [P, 2 * QTS], F32, name=tg + "_t",
                         tag=tg, bufs=2, padded_shape=[P, 2 * QTS])

    # ---------------- phase A unit: projections + RoPE for one token tile
    def emit_proj(tt):
        ts0 = tt * T_TILE
        if tt in xt_tiles:
            xt = xt_tiles[tt]
        else:
            xt = xpool.tile([P, EC, T_TILE], BF16, tag="xt")
            for q4 in range(4):
                nc.sync.dma_start(xt[:, q4sl[q4], :], xTt[tt, :, q4sl[q4], :])
        # rope map slices for this token range
        nc.sync.dma_start(m1_s[:, ts0:ts0 + T_TILE], rm1[:, ts0:ts0 + T_TILE])
        nc.sync.dma_start(m2_s[:, ts0:ts0 + T_TILE], rm2[:, ts0:ts0 + T_TILE])

        pools = [psA, psO, psA, psO]  # alternate pools -> 4-deep rotation
        for gi, (w_s, dst) in enumerate(((wq_s, qT_s), (wk_s, kT_s))):
            psb = _ps_tile(pools[gi])
            for hs in range(HPC):
                ps = psb[:, hs * T_TILE:(hs + 1) * T_TILE]
                for ec in range(EC):
                    nc.tensor.matmul(
                        ps,
                        lhsT=w_s[:, ec, hs * P:(hs + 1) * P],
                        rhs=xt[:, ec, :],
                        start=(ec == 0),
                        stop=(ec == EC - 1),
                    )
                # RoPE: e = [x1; x2], swp = [x2; x1] (half-swap via DMA);
                # out = e*[cos;cos] + swp*[-sin;sin]
                e_t = work.tile([P, T_TILE], BF16, tag="rope_e")
                nc.scalar.copy(e_t[:], ps)
                swp = work.tile([P, T_TILE], BF16, tag="rope_s")
                nc.sync.dma_start(swp[0:64, :], e_t[64:128, :])
                nc.sync.dma_start(swp[64:128, :], e_t[0:64, :])
                a_t = work.tile([P, T_TILE], BF16, tag="rope_a")
                b_t = work.tile([P, T_TILE], BF16, tag="rope_b")
                nc.vector.tensor_mul(a_t[:], e_t[:], m1_s[:, ts0:ts0 + T_TILE])
                nc.vector.tensor_mul(b_t[:], swp[:], m2_s[:, ts0:ts0 + T_TILE])
                nc.vector.tensor_add(dst[:, hs, ts0:ts0 + T_TILE], a_t[:], b_t[:])

        for sp in range(T_TILE // P // 2):
            psb = _ps_tile(pools[2 + sp], shape=[P, 2 * HD])
            for k in range(2):
                sub = 2 * sp + k
                for ec in range(EC):
                    nc.tensor.matmul(
                        psb[:, k * HD:(k + 1) * HD],
                        lhsT=xt[:, ec, sub * P:(sub + 1) * P],
                        rhs=wv_s[:, ec, :],
                        start=(ec == 0),
                        stop=(ec == EC - 1),
                    )
            nc.scalar.copy(
                v_s[:, tt * (T_TILE // P) + 2 * sp:
                    tt * (T_TILE // P) + 2 * sp + 2, :], psb[:])

    # ---------------- phase B unit: attention for one (batch, q-tile)
    def emit_attn(b, qt):
        q0 = qt * QTS
        nck = (q0 + QTS) // P  # causal: key chunks 0..nck-1
        for hs in range(HPC):
            qTb = qT_s[:, hs, b * S:(b + 1) * S]
            kTb = kT_s[:, hs, b * S:(b + 1) * S]
            # one 2-bank tile per q-tile: left bank = PV out^T accumulator,
            # right bank = denominator (ones-matmul); double-buffered so
            # consecutive q-tile chains overlap
            opd = _ps_tile(psO)
            ops = opd[:, 0:QTS]
            dps = opd[:, QTS:2 * QTS]
            for pp in range(nck // 2):
                cc = (2 * pp, 2 * pp + 1)
                # causal trim: diagonal chunk j (=c-(nck-4)) only has
                # valid queries q >= q0 + 128*j  ->  width 512-128*j
                jj = [max(0, c - (nck - 4)) for c in cc]
                off = [128 * j for j in jj]
                sps = _ps_tile(psA)
                for half, c in enumerate(cc):
                    nc.tensor.matmul(
                        sps[:, half * QTS + off[half]:(half + 1) * QTS],
                        lhsT=kTb[:, c * P:(c + 1) * P],
                        rhs=qTb[:, q0 + off[half]:q0 + QTS],
                        start=True,
                        stop=True,
                    )
                ex = work.tile([P, 2 * QTS], BF16, tag="exps", bufs=6)
                if off[0] == 0:
                    # single full-width exp; any garbage in half 1's
                    # trimmed zone is finite and never consumed
                    nc.scalar.activation(
                        ex[:], sps[:], mybir.ActivationFunctionType.Exp,
                        scale=ATTN_SCALE,
                    )
                else:
                    for half in range(2):
                        sl = slice(half * QTS + off[half], (half + 1) * QTS)
                        nc.scalar.activation(
                            ex[:, sl], sps[:, sl],
                            mybir.ActivationFunctionType.Exp,
                            scale=ATTN_SCALE,
                        )
                for half, c in enumerate(cc):
                    w = QTS - off[half]
                    exh = ex[:, half * QTS + off[half]:(half + 1) * QTS]
                    if c >= nck - 4:
                        # intra-block triangle: reuse the j=0 mask, width w
                        nc.vector.tensor_mul(exh, exh, mk_s[:, 0, :w])
                    nc.tensor.matmul(
                        ops[:, off[half]:QTS],
                        lhsT=v_s[:, b * NKC + c, hs * P:(hs + 1) * P],
                        rhs=exh,
                        start=(c == 0),
                        stop=(c == nck - 1),
                    )
                    nc.tensor.matmul(
                        dps[:, off[half]:QTS],
                        lhsT=ones_s[:],
                        rhs=exh,
                        start=(c == 0),
                        stop=(c == nck - 1),
                    )
            # normalize: oT = ops * (1/denom), denom replicated to all
            # 128 partitions by the ones-matmul
            oslice = oT_s[:, hs, b * S + q0: b * S + q0 + QTS]
            rb = work.tile([P, QTS], F32, tag="recipb")
            nc.vector.reciprocal_approx_fast(out=rb[:], in_=dps)
            nc.vector.tensor_mul(oslice, ops, rb[:])

    # ---------------- phase C unit: output projection for one batch
    def emit_outproj(b, last):
        for tch in range(S // P):
            t0 = b * S + tch * P
            stage = work.tile([P, E], BF16 if OUT_BF16 else F32, tag="wo_out")
            # hc-outer: one stationary (oT chunk) streams all 4 E-slices
            # before switching, consolidating the PE weight stream
            wps = [_ps_tile(psA, shape=[P, 1024]),
                   _ps_tile(psO, shape=[P, 1024])]
            for hc in range(HPC):
                for es in range(4):
                    nc.tensor.matmul(
                        wps[es // 2][:, (es % 2) * 512:(es % 2 + 1) * 512],
                        lhsT=oT_s[:, hc, t0:t0 + P],
                        rhs=wo_s[:, hc, es * 512:(es + 1) * 512],
                        start=(hc == 0),
                        stop=(hc == HPC - 1),
                    )
            for ep in range(2):
                # evictions alternate DVE/ACT so neither engine gates PE
                if ep == 0:
                    nc.vector.tensor_copy(
                        out=stage[:, 0:1024], in_=wps[0][:])
                else:
                    nc.scalar.copy(stage[:, 1024:2048], wps[1][:])
                if last and tch == S // P - 1:
                    # drain the final tile per-slice to shorten the tail
                    nc.sync.dma_start(
                        out[t0:t0 + P, ep * 1024:(ep + 1) * 1024],
                        stage[:, ep * 1024:(ep + 1) * 1024])
            if not (last and tch == S // P - 1):
                nc.sync.dma_start(out[t0:t0 + P, :], stage[:])

    # ---------------- emission schedule: A, B0, B1, C0, C1.  B1's scores
    # only need phase A, so they fill B0's exp-wait gaps; C0's chunks
    # become ready per-q-tile during B and fill B1's; the ACT exp stream
    # runs continuously across B0+B1.  (Fine-grained A/B interleaving was
    # measured WORSE — PE sequencer wait-queue starvation + HAM.)
    for tt in range(NT):
        emit_proj(tt)
        if tt == 1:
            nc.sync.dma_start(wo_s[:], woT[:])
    for b in range(B):
        for qt in range(NQT):
            emit_attn(b, qt)
    emit_outproj(0, last=False)
    emit_outproj(1, last=True)


def build_nc():
    nc = bacc.Bacc("TRN2", target_bir_lowering=False, debug=False, num_devices=1)
    with tile.TileContext(nc) as tc, ExitStack() as ctx:
        _emit(tc, ctx)
    nc.compile()
    return nc


# ---------------------------------------------------------------- host prep
def _rope_maps():
    half = D // 2
    inv = 1.0 / (ROPE_BASE ** (np.arange(half, dtype=np.float64) / half))
    ang = np.arange(S, dtype=np.float64)[None, :] * inv[:, None]  # [64, S]
    cos = np.cos(ang)
    sin = np.sin(ang)
    m1 = np.concatenate([cos, cos], axis=0)   # [128, S] multiplies e=[x1;x2]
    m2 = np.concatenate([-sin, sin], axis=0)  # multiplies swp=[x2;x1]
    m1 = np.tile(m1, (1, B)).astype(NPBF16)   # [128, T] (t = b*S + s)
    m2 = np.tile(m2, (1, B)).astype(NPBF16)
    return np.ascontiguousarray(m1), np.ascontiguousarray(m2)


def _masks():
    kk = np.arange(P)[:, None]
    qq = np.arange(QTS)[None, :]
    m = np.stack([(kk + 128 * j <= qq) for j in range(4)], axis=1)
    return np.ascontiguousarray(m.astype(NPBF16))  # [128, 4, 512]


def _prep_in_maps(x, Wq, Wk, Wv, Wo):
    x = np.asarray(x, np.float32)
    Wq = np.asarray(Wq, np.float32)
    Wk = np.asarray(Wk, np.float32)
    Wv = np.asarray(Wv, np.float32)
    Wo = np.asarray(Wo, np.float32)

    # x^T tiled: [NT, 128, EC, T_TILE];  xT[e, t] = x[t, e]
    xT = x.reshape(T, E).T.astype(NPBF16)                      # [E, T]
    xtt = xT.reshape(EC, P, NT, T_TILE).transpose(2, 1, 0, 3)  # [NT,P,EC,TT]
    xtt = np.ascontiguousarray(xtt)

    m1, m2 = _rope_maps()
    msk = _masks()

    # de-interleave perm for RoPE pair-contiguity
    perm = np.concatenate([np.arange(0, D, 2), np.arange(1, D, 2)])

    def wslice(W, rows):
        # -> [P, EC, ncols] : wT[p, ec, c] = W[rows[c], ec*128 + p]
        wt = W[rows].T.astype(NPBF16)            # [E, ncols]
        return np.ascontiguousarray(
            wt.reshape(EC, P, len(rows)).transpose(1, 0, 2))

    in_maps = []
    for core in range(N_CORES):
        heads = range(core * HPC, (core + 1) * HPC)
        rows_qk = np.concatenate([h * D + perm for h in heads])
        rows_v = np.concatenate([np.arange(h * D, (h + 1) * D) for h in heads])
        # woT[p, hc, e] = Wo[e, rows_v[hc*128 + p]]
        wo_t = Wo[:, rows_v].T.astype(NPBF16)    # [HD, E]
        wo_t = np.ascontiguousarray(
            wo_t.reshape(HPC, P, E).transpose(1, 0, 2))
        in_maps.append({
            "xTt": xtt,
            "wqT": wslice(Wq, rows_qk),
            "wkT": wslice(Wk, rows_qk),
            "wvT": wslice(Wv, rows_v),
            "woT": wo_t,
            "rm1": m1,
            "rm2": m2,
            "msk": msk,
        })
    return in_maps


_NC_CACHE = None


def _get_nc():
    global _NC_CACHE
    if _NC_CACHE is None:
        _NC_CACHE = build_nc()
    return _NC_CACHE


def kernel(x, Wq, Wk, Wv, Wo, _want_trace=False):
    in_maps = _prep_in_maps(x, Wq, Wk, Wv, Wo)
    nc = _get_nc()
    trace = _want_trace or bool(os.environ.get("KERNEL_TRACE"))
    res = bass_utils.run_bass_kernel_spmd(
        nc, in_maps, core_ids=list(range(N_CORES)), trace=trace,
    )
    acc = np.zeros((T, E), np.float64)
    for c in range(N_CORES):
        acc += res.results[c]["out"].astype(np.float64)
    outv = acc.astype(np.float32).reshape(B, S, E)
    if _want_trace:
        return outv, res
    return outv


# revision 33
# speedup vs baseline: 1.2142x; 1.0066x over previous
"""Tensor-parallel causal multi-head attention (RoPE) for 8 Trainium2 cores.

Problem: nn_Attention (B=2, S=2048, E=2048, H=16, interleaved-pair RoPE,
causal softmax with 1/sqrt(E) scaling, output projection).

Sharding: tensor-parallel over heads — each of the 8 cores owns 2 heads
(the matching 256 columns of Wq/Wk/Wv and rows of Wo), x is replicated,
and the post-Wo all-reduce is done on the host (sum of 8 partials).

Per-core device pipeline (all matmuls bf16, fp32 accumulation):
  1. QK^T projections in transposed layout  Q^T/K^T [D, t]  (tokens on the
     free axis), V in natural layout [t, D].  RoPE is applied to Q^T/K^T on
     the vector engine using host-precomputed cos/sin maps; the head-dim is
     de-interleaved (even feats then odd feats) via a host-side permutation
     of the Wq/Wk rows so the rotation pairs are contiguous partitions.
  2. Attention per (batch, head) over q-tiles of 512 with 128-wide key
     chunks processed in pairs (one [128,1024] exp per pair on ACT when the
     pair is untrimmed or first-diagonal, scale folded in; no
     max-subtraction: |scores/sqrt(E)| <~ 1.5 for these inputs).  Diagonal
     chunks are causally trimmed to their valid N = 512-128*j query range
     and masked with the sliced j=0 triangle mask (DVE).  PV accumulates
     out^T += V_c^T probs^T  on PE into the left bank of a 2-bank psO
     tile; the denominator rides on PE as a ones[128,128]-stationary
     matmul into the right bank.
  3. Normalization: reciprocal_approx_fast(denom) then one DVE multiply
     fused into the out^T psum eviction.
  4. Output projection from out^T (stationary) -> bf16 partial [t, E],
     staged per token-chunk (evictions alternate DVE/ACT so neither engine
     gates PE) and written back with one DMA.  Host sums the 8 partials
     in fp64.

Scheduling notes (measured, HW exec ~350 us vs 385 us predecessor):
  - Phase emission A(0..7), B0, B1, C0, C1: B1's scores depend only on A
    so they fill B0's exp-wait gaps; C0's chunks become ready per-q-tile
    during B and fill B1's; the ACT exp stream runs continuously.
  - Fine-grained A/B interleaving and moving the denominator off PE both
    measured WORSE: cross-engine deps starve the PE sequencer's 4-deep
    wait queue (exposed LDWEIGHTS + HAM clock-gate oscillation, up to
    +19%% on matmul pacing).
  - PSUM as two tags x 2 bufs of [128,1024] f32 (all 8 banks): phases A/C
    alternate tags for a 4-deep matmul-psum rotation; phase B uses one
    tag for the double-buffered PV/denominator q-tile chains (consecutive
    chains overlap) and the other for score pairs.
  - Startup: coalesced ~0.5 MB DMAs ordered wq -> x(0) -> wk -> wv (the
    single HWDGE ring is the startup bottleneck), RoPE maps sliced
    per-tile, Wo deferred to mid-phase-A; 48 dummy matmuls bridge the
    startup DMA window so the PE HAM clock stays warm.
"""

import math
import os
from contextlib import ExitStack

import ml_dtypes
import numpy as np

import concourse.bass as bass
import concourse.mybir as mybir
import concourse.tile as tile
from concourse import bacc, bass_isa, bass_utils

# partial-output dtype: bf16 halves the output DMA; host sums in fp64
OUT_BF16 = os.environ.get("KERNEL_OUT", "bf16") == "bf16"

# ---------------------------------------------------------------- constants
B, S, E = 2, 2048, 2048
H = 16
N_CORES = 8
HPC = H // N_CORES          # heads per core = 2
D = E // H                  # head dim = 128
T = B * S                   # tokens = 4096
HD = HPC * D                # per-core head dims = 256
ATTN_SCALE = 1.0 / math.sqrt(E)
ROPE_BASE = 10000.0

P = 128
EC = E // P                 # 16 contraction chunks
T_TILE = 512
NT = T // T_TILE            # 8 projection token tiles
QTS = 512                   # attention q-tile size
NQT = S // QTS              # 4 q-tiles per (b, h)
NKC = S // P                # 16 key chunks per batch
N_WARM = 96                 # HAM warm-up matmuls bridging the startup DMAs

BF16 = mybir.dt.bfloat16
F32 = mybir.dt.float32
NPBF16 = ml_dtypes.bfloat16


# ---------------------------------------------------------------- device IR
def _emit(tc, ctx):
    nc = tc.nc
    xTt = nc.dram_tensor("xTt", [NT, P, EC, T_TILE], BF16, kind="ExternalInput").ap()
    wqT = nc.dram_tensor("wqT", [P, EC, HD], BF16, kind="ExternalInput").ap()
    wkT = nc.dram_tensor("wkT", [P, EC, HD], BF16, kind="ExternalInput").ap()
    wvT = nc.dram_tensor("wvT", [P, EC, HD], BF16, kind="ExternalInput").ap()
    woT = nc.dram_tensor("woT", [P, HPC, E], BF16, kind="ExternalInput").ap()
    rm1 = nc.dram_tensor("rm1", [P, T], BF16, kind="ExternalInput").ap()
    rm2 = nc.dram_tensor("rm2", [P, T], BF16, kind="ExternalInput").ap()
    msk = nc.dram_tensor("msk", [P, 4, QTS], BF16, kind="ExternalInput").ap()
    out = nc.dram_tensor("out", [T, E], BF16 if OUT_BF16 else F32,
                         kind="ExternalOutput").ap()

    wpool = ctx.enter_context(tc.tile_pool(name="wpool", bufs=1))
    xpool = ctx.enter_context(tc.tile_pool(name="xpool", bufs=2))
    qkv = ctx.enter_context(tc.tile_pool(name="qkv", bufs=1))
    work = ctx.enter_context(tc.tile_pool(name="work", bufs=3))
    psA = ctx.enter_context(tc.tile_pool(name="psA", bufs=2, space="PSUM"))
    psO = ctx.enter_context(tc.tile_pool(name="psO", bufs=2, space="PSUM"))

    # --- persistent SBUF state
    wq_s = wpool.tile([P, EC, HD], BF16)
    wk_s = wpool.tile([P, EC, HD], BF16)
    wv_s = wpool.tile([P, EC, HD], BF16)
    wo_s = wpool.tile([P, HPC, E], BF16)
    m1_s = wpool.tile([P, T], BF16)
    m2_s = wpool.tile([P, T], BF16)
    mk_s = wpool.tile([P, 4, QTS], BF16)
    ones_s = wpool.tile([P, P], BF16)
    nc.any.memset(ones_s[:], 1.0)

    # startup-latency ordering: feed the first Q-group's operands (all of
    # wq + x tile 0) ahead of everything else, then wk, wv, masks.
    q4sl = [slice(q * (EC // 4), (q + 1) * (EC // 4)) for q in range(4)]
    # coalesced transfers (~0.5 MB each) — per-DMA ring overhead is what
    # delays the later weights, so fewer/bigger beats many/small here
    nc.sync.dma_start(wq_s[:, 0:8, :], wqT[:, 0:8, :])
    xt0 = xpool.tile([P, EC, T_TILE], BF16, tag="xt")
    nc.sync.dma_start(xt0[:, 0:4, :], xTt[0, :, 0:4, :])
    nc.sync.dma_start(wq_s[:, 8:16, :], wqT[:, 8:16, :])
    nc.sync.dma_start(xt0[:, 4:8, :], xTt[0, :, 4:8, :])
    # HAM warm-up: dummy matmuls bridging the startup DMA window so the
    # first real matmuls run at the full 2.4 GHz clock
    warm = psA.tile([P, 512], F32, tag="big", bufs=2,
                    padded_shape=[P, 2 * QTS])
    for i in range(N_WARM):
        nc.tensor.matmul(warm[:, 0:P], lhsT=ones_s[:], rhs=ones_s[:],
                         start=(i == 0), stop=(i == N_WARM - 1))
    nc.sync.dma_start(xt0[:, 8:12, :], xTt[0, :, 8:12, :])
    nc.sync.dma_start(xt0[:, 12:16, :], xTt[0, :, 12:16, :])
    for h2 in range(2):
        nc.sync.dma_start(wk_s[:, 8 * h2:8 * (h2 + 1), :],
                          wkT[:, 8 * h2:8 * (h2 + 1), :])
    for h2 in range(2):
        nc.sync.dma_start(wv_s[:, 8 * h2:8 * (h2 + 1), :],
                          wvT[:, 8 * h2:8 * (h2 + 1), :])
    nc.sync.dma_start(mk_s[:], msk[:])

    qT_s = qkv.tile([P, HPC, T], BF16)   # roped Q^T  [d, h, t]
    kT_s = qkv.tile([P, HPC, T], BF16)   # roped K^T
    v_s = qkv.tile([P, T // P, HD], BF16)  # V natural [t%128, t//128, hd]
    oT_s = qkv.tile([P, HPC, T], BF16)   # normalized out^T [d, h, t]

    xt_tiles = {0: xt0}

    # two psum tags, each 2 bufs of [P, 1024] f32 (2 banks) = all 8 banks;
    # phases A/C alternate tags for a 4-deep matmul-psum rotation, phase B
    # uses "outT" for the per-q-tile PV/denominator chains and "big" for
    # score pairs
    def _ps_tile(pool, shape=None):
        tg = "big" if pool is psA else "outT"
        return pool.tile(shape or [P, 2 * QTS], F32, name=tg + "_t",
                         tag=tg, bufs=2, padded_shape=[P, 2 * QTS])

    # ---------------- phase A unit: projections + RoPE for one token tile
    def emit_proj(tt):
        ts0 = tt * T_TILE
        if tt in xt_tiles:
            xt = xt_tiles[tt]
        else:
            xt = xpool.tile([P, EC, T_TILE], BF16, tag="xt")
            for q4 in range(4):
                nc.sync.dma_start(xt[:, q4sl[q4], :], xTt[tt, :, q4sl[q4], :])
        # rope map slices for this token range
        nc.sync.dma_start(m1_s[:, ts0:ts0 + T_TILE], rm1[:, ts0:ts0 + T_TILE])
        nc.sync.dma_start(m2_s[:, ts0:ts0 + T_TILE], rm2[:, ts0:ts0 + T_TILE])

        pools = [psA, psO, psA, psO]  # alternate pools -> 4-deep rotation
        for gi, (w_s, dst) in enumerate(((wq_s, qT_s), (wk_s, kT_s))):
            psb = _ps_tile(pools[gi])
            for hs in range(HPC):
                ps = psb[:, hs * T_TILE:(hs + 1) * T_TILE]
                for ec in range(EC):
                    nc.tensor.matmul(
                        ps,
                        lhsT=w_s[:, ec, hs * P:(hs + 1) * P],
                        rhs=xt[:, ec, :],
                        start=(ec == 0),
                        stop=(ec == EC - 1),
                    )
                # RoPE: e = [x1; x2], swp = [x2; x1] (half-swap via DMA);
                # out = e*[cos;cos] + swp*[-sin;sin]
                e_t = work.tile([P, T_TILE], BF16, tag="rope_e")
                nc.scalar.copy(e_t[:], ps)
                swp = work.tile([P, T_TILE], BF16, tag="rope_s")
                nc.sync.dma_start(swp[0:64, :], e_t[64:128, :])
                nc.sync.dma_start(swp[64:128, :], e_t[0:64, :])
                a_t = work.tile([P, T_TILE], BF16, tag="rope_a")
                b_t = work.tile([P, T_TILE], BF16, tag="rope_b")
                nc.vector.tensor_mul(a_t[:], e_t[:], m1_s[:, ts0:ts0 + T_TILE])
                nc.vector.tensor_mul(b_t[:], swp[:], m2_s[:, ts0:ts0 + T_TILE])
                nc.vector.tensor_add(dst[:, hs, ts0:ts0 + T_TILE], a_t[:], b_t[:])

        for sp in range(T_TILE // P // 2):
            psb = _ps_tile(pools[2 + sp], shape=[P, 2 * HD])
            for k in range(2):
                sub = 2 * sp + k
                for ec in range(EC):
                    nc.tensor.matmul(
                        psb[:, k * HD:(k + 1) * HD],
                        lhsT=xt[:, ec, sub * P:(sub + 1) * P],
                        rhs=wv_s[:, ec, :],
                        start=(ec == 0),
                        stop=(ec == EC - 1),
                    )
            nc.scalar.copy(
                v_s[:, tt * (T_TILE // P) + 2 * sp:
                    tt * (T_TILE // P) + 2 * sp + 2, :], psb[:])

    # ---------------- phase B unit: attention for one (batch, q-tile)
    def emit_attn(b, qt):
        q0 = qt * QTS
        nck = (q0 + QTS) // P  # causal: key chunks 0..nck-1
        for hs in range(HPC):
            qTb = qT_s[:, hs, b * S:(b + 1) * S]
            kTb = kT_s[:, hs, b * S:(b + 1) * S]
            # one 2-bank tile per q-tile: left bank = PV out^T accumulator,
            # right bank = denominator (ones-matmul); double-buffered so
            # consecutive q-tile chains overlap
            opd = _ps_tile(psO)
            ops = opd[:, 0:QTS]
            dps = opd[:, QTS:2 * QTS]
            for pp in range(nck // 2):
                cc = (2 * pp, 2 * pp + 1)
                # causal trim: diagonal chunk j (=c-(nck-4)) only has
                # valid queries q >= q0 + 128*j  ->  width 512-128*j
                jj = [max(0, c - (nck - 4)) for c in cc]
                off = [128 * j for j in jj]
                sps = _ps_tile(psA)
                for half, c in enumerate(cc):
                    nc.tensor.matmul(
                        sps[:, half * QTS + off[half]:(half + 1) * QTS],
                        lhsT=kTb[:, c * P:(c + 1) * P],
                        rhs=qTb[:, q0 + off[half]:q0 + QTS],
                        start=True,
                        stop=True,
                    )
                ex = work.tile([P, 2 * QTS], BF16, tag="exps", bufs=6)
                if off[0] == 0:
                    # single full-width exp; any garbage in half 1's
                    # trimmed zone is finite and never consumed
                    nc.scalar.activation(
                        ex[:], sps[:], mybir.ActivationFunctionType.Exp,
                        scale=ATTN_SCALE,
                    )
                else:
                    for half in range(2):
                        sl = slice(half * QTS + off[half], (half + 1) * QTS)
                        nc.scalar.activation(
                            ex[:, sl], sps[:, sl],
                            mybir.ActivationFunctionType.Exp,
                            scale=ATTN_SCALE,
                        )
                for half, c in enumerate(cc):
                    w = QTS - off[half]
                    exh = ex[:, half * QTS + off[half]:(half + 1) * QTS]
                    if c >= nck - 4:
                        # intra-block triangle: reuse the j=0 mask, width w
                        nc.vector.tensor_mul(exh, exh, mk_s[:, 0, :w])
                    nc.tensor.matmul(
                        ops[:, off[half]:QTS],
                        lhsT=v_s[:, b * NKC + c, hs * P:(hs + 1) * P],
                        rhs=exh,
                        start=(c == 0),
                        stop=(c == nck - 1),
                    )
                    nc.tensor.matmul(
                        dps[:, off[half]:QTS],
                        lhsT=ones_s[:],
                        rhs=exh,
                        start=(c == 0),
                        stop=(c == nck - 1),
                    )
            # normalize: oT = ops * (1/denom), denom replicated to all
            # 128 partitions by the ones-matmul
            oslice = oT_s[:, hs, b * S + q0: b * S + q0 + QTS]
            rb = work.tile([P, QTS], F32, tag="recipb")
            nc.vector.reciprocal_approx_fast(out=rb[:], in_=dps)
            nc.vector.tensor_mul(oslice, ops, rb[:])

    # ---------------- phase C unit: output projection for one batch
    def emit_outproj(b, last):
        for tch in range(S // P):
            t0 = b * S + tch * P
            stage = work.tile([P, E], BF16 if OUT_BF16 else F32, tag="wo_out")
            # hc-outer: one stationary (oT chunk) streams all 4 E-slices
            # before switching, consolidating the PE weight stream
            wps = [_ps_tile(psA, shape=[P, 1024]),
                   _ps_tile(psO, shape=[P, 1024])]
            for hc in range(HPC):
                for es in range(4):
                    nc.tensor.matmul(
                        wps[es // 2][:, (es % 2) * 512:(es % 2 + 1) * 512],
                        lhsT=oT_s[:, hc, t0:t0 + P],
                        rhs=wo_s[:, hc, es * 512:(es + 1) * 512],
                        start=(hc == 0),
                        stop=(hc == HPC - 1),
                    )
            for ep in range(2):
                # evictions alternate DVE/ACT so neither engine gates PE
                if ep == 0:
                    nc.vector.tensor_copy(
                        out=stage[:, 0:1024], in_=wps[0][:])
                else:
                    nc.scalar.copy(stage[:, 1024:2048], wps[1][:])
                if last and tch == S // P - 1:
                    # drain the final tile per-slice to shorten the tail
                    nc.sync.dma_start(
                        out[t0:t0 + P, ep * 1024:(ep + 1) * 1024],
                        stage[:, ep * 1024:(ep + 1) * 1024])
            if not (last and tch == S // P - 1):
                nc.sync.dma_start(out[t0:t0 + P, :], stage[:])

    # ---------------- emission schedule: A, B0, B1, C0, C1.  B1's scores
    # only need phase A, so they fill B0's exp-wait gaps; C0's chunks
    # become ready per-q-tile during B and fill B1's; the ACT exp stream
    # runs continuously across B0+B1.  (Fine-grained A/B interleaving was
    # measured WORSE — PE sequencer wait-queue starvation + HAM.)
    for tt in range(NT):
        emit_proj(tt)
        if tt == 1:
            nc.sync.dma_start(wo_s[:], woT[:])
    for b in range(B):
        for qt in range(NQT):
            emit_attn(b, qt)
    emit_outproj(0, last=False)
    emit_outproj(1, last=True)


def build_nc():
    nc = bacc.Bacc("TRN2", target_bir_lowering=False, debug=False, num_devices=1)
    with tile.TileContext(nc) as tc, ExitStack() as ctx:
        _emit(tc, ctx)
    nc.compile()
    return nc


# ---------------------------------------------------------------- host prep
def _rope_maps():
    half = D // 2
    inv = 1.0 / (ROPE_BASE ** (np.arange(half, dtype=np.float64) / half))
    ang = np.arange(S, dtype=np.float64)[None, :] * inv[:, None]  # [64, S]
    cos = np.cos(ang)
    sin = np.sin(ang)
    m1 = np.concatenate([cos, cos], axis=0)   # [128, S] multiplies e=[x1;x2]
    m2 = np.concatenate([-sin, sin], axis=0)  # multiplies swp=[x2;x1]
    m1 = np.tile(m1, (1, B)).astype(NPBF16)   # [128, T] (t = b*S + s)
    m2 = np.tile(m2, (1, B)).astype(NPBF16)
    return np.ascontiguousarray(m1), np.ascontiguousarray(m2)


def _masks():
    kk = np.arange(P)[:, None]
    qq = np.arange(QTS)[None, :]
    m = np.stack([(kk + 128 * j <= qq) for j in range(4)], axis=1)
    return np.ascontiguousarray(m.astype(NPBF16))  # [128, 4, 512]


def _prep_in_maps(x, Wq, Wk, Wv, Wo):
    x = np.asarray(x, np.float32)
    Wq = np.asarray(Wq, np.float32)
    Wk = np.asarray(Wk, np.float32)
    Wv = np.asarray(Wv, np.float32)
    Wo = np.asarray(Wo, np.float32)

    # x^T tiled: [NT, 128, EC, T_TILE];  xT[e, t] = x[t, e]
    xT = x.reshape(T, E).T.astype(NPBF16)                      # [E, T]
    xtt = xT.reshape(EC, P, NT, T_TILE).transpose(2, 1, 0, 3)  # [NT,P,EC,TT]
    xtt = np.ascontiguousarray(xtt)

    m1, m2 = _rope_maps()
    msk = _masks()

    # de-interleave perm for RoPE pair-contiguity
    perm = np.concatenate([np.arange(0, D, 2), np.arange(1, D, 2)])

    def wslice(W, rows):
        # -> [P, EC, ncols] : wT[p, ec, c] = W[rows[c], ec*128 + p]
        wt = W[rows].T.astype(NPBF16)            # [E, ncols]
        return np.ascontiguousarray(
            wt.reshape(EC, P, len(rows)).transpose(1, 0, 2))

    in_maps = []
    for core in range(N_CORES):
        heads = range(core * HPC, (core + 1) * HPC)
        rows_qk = np.concatenate([h * D + perm for h in heads])
        rows_v = np.concatenate([np.arange(h * D, (h + 1) * D) for h in heads])
        # woT[p, hc, e] = Wo[e, rows_v[hc*128 + p]]
        wo_t = Wo[:, rows_v].T.astype(NPBF16)    # [HD, E]
        wo_t = np.ascontiguousarray(
            wo_t.reshape(HPC, P, E).transpose(1, 0, 2))
        in_maps.append({
            "xTt": xtt,
            "wqT": wslice(Wq, rows_qk),
            "wkT": wslice(Wk, rows_qk),
            "wvT": wslice(Wv, rows_v),
            "woT": wo_t,
            "rm1": m1,
            "rm2": m2,
            "msk": msk,
        })
    return in_maps


_NC_CACHE = None


def _get_nc():
    global _NC_CACHE
    if _NC_CACHE is None:
        _NC_CACHE = build_nc()
    return _NC_CACHE


def kernel(x, Wq, Wk, Wv, Wo, _want_trace=False):
    in_maps = _prep_in_maps(x, Wq, Wk, Wv, Wo)
    nc = _get_nc()
    trace = _want_trace or bool(os.environ.get("KERNEL_TRACE"))
    res = bass_utils.run_bass_kernel_spmd(
        nc, in_maps, core_ids=list(range(N_CORES)), trace=trace,
    )
    acc = np.zeros((T, E), np.float64)
    for c in range(N_CORES):
        acc += res.results[c]["out"].astype(np.float64)
    outv = acc.astype(np.float32).reshape(B, S, E)
    if _want_trace:
        return outv, res
    return outv
